# revision 1
# baseline (speedup 1.0000x reference)
"""PairEmbedding Bass kernel for 8 TRN2 NeuronCores.

out[b,i,j,:] = Co[b,j,:] + Cp[b,i,:] + sep(b,i,j) * w_sep
  Co[j] = se_j @ W1 + [0 | pe_j]
  Cp[i] = se_i @ W2 + b_proj + [pe_i | 0]
  sep(i,j) = ln(|aa_i - aa_j| + 1)
where se = emb_table[seq], pe = pos_table[aa_idx], W1 = W_proj[0:144],
W2 = W_proj[144:288], w_sep = W_proj[288].

Sharding: core c -> batch b = c//2, row block i in [128*(c%2), 128*(c%2)+128),
all 256 j. Per-core output (128, 256, 288) stored bf16, host upcasts to f32
(bf16 rounding ~0.4% << the 2e-2 gate).

Per-j work (j-loop), spread across all five engines via a pattern mix
(default P:64,Q:64,V:96,W:32, tuned on HW):
  P: PE MM_A(co bcast)+MM_B(sep x wsep); DVE   ob = ps + cp
  Q: PE MM_A+MM_B;                       ACT   ob = Iden(ps); GPS ob += cp
  V: PE MM_A;  DVE ob = wsep*sep_j + ps; GPS   ob += cp
  W: PE MM_A;  DVE ob = wsep*sep_j + ps; DVE   ob += cp
  DMA: groups of JG=32 j's (2.36 MB) alternating sync/scalar HWDGE queues.
Matmul operands are indexed along the free dim from base partitions {0, 64}
(HW requires base partition in {0,32,64}).
"""

import math
from contextlib import ExitStack

import numpy as np

from concourse import bacc, bass, mybir, tile
from concourse.bass_utils import run_bass_kernel_spmd

dt = mybir.dt
AF = mybir.ActivationFunctionType
ALU = mybir.AluOpType

B = 4
L = 256
D_PAIR = 288
D_HALF = 144
MAX_LEN = 260
VOCAB = 21
IH = 128          # i rows per core
JG = 32           # j's per output DMA group
N_CORES = 8


def _pos_enc_table() -> np.ndarray:
    idx = np.arange(0, D_HALF, 2, dtype=np.float32)
    t = (np.float32(math.log(10000.0)) * idx) / np.float32(D_HALF)
    denom = np.exp(t, dtype=np.float32)
    pos = np.arange(MAX_LEN, dtype=np.float32)[:, None]
    pe = np.zeros((MAX_LEN, D_HALF), dtype=np.float32)
    pe[:, 0::2] = np.sin(pos / denom, dtype=np.float32)
    pe[:, 1::2] = np.cos(pos / denom, dtype=np.float32)
    return pe


def _bcast(ap_src, nparts: int):
    return bass.AP(
        tensor=ap_src.tensor, offset=ap_src.offset, ap=[[0, nparts], *ap_src.ap]
    )


def _pattern_seq(pat: str):
    """'P:64,Q:64,V:96,W:32' -> interleaved length-256 list of pattern codes."""
    counts = {}
    for part in pat.split(","):
        k, v = part.split(":")
        counts[k.strip()] = int(v)
    assert sum(counts.values()) == L
    seq = []
    used = {k: 0 for k in counts}
    for i in range(1, L + 1):
        k = max(counts, key=lambda c: counts[c] * i / L - used[c])
        seq.append(k)
        used[k] += 1
    return seq


def build(
    stage: str = "full",
    repeat: int = 1,
    variant: str = "",
    jg: int = JG,
    pat: str = "P:64,Q:64,V:96,W:32",
    obufs: int = 2,
) -> bass.Bass:
    nc = bacc.Bacc("TRN2", target_bir_lowering=False)

    seqb_d = nc.dram_tensor("seqb", [L], dt.int32, kind="ExternalInput")
    seqi_d = nc.dram_tensor("seqi", [IH], dt.int32, kind="ExternalInput")
    aab_d = nc.dram_tensor("aab", [L], dt.int32, kind="ExternalInput")
    aai_d = nc.dram_tensor("aai", [IH], dt.int32, kind="ExternalInput")
    emb_d = nc.dram_tensor("emb", [VOCAB, D_HALF], dt.float32, kind="ExternalInput")
    wp_d = nc.dram_tensor("wp", [D_PAIR + 1, D_PAIR], dt.float32, kind="ExternalInput")
    bp_d = nc.dram_tensor("bp", [D_PAIR], dt.float32, kind="ExternalInput")
    out_d = nc.dram_tensor("out", [IH, L, D_PAIR], dt.bfloat16, kind="ExternalOutput")

    # pos-table gather sources, pre-arranged on host: chunk c of <=128 pos
    # rows on partitions, channel slice [0:144] (posL, pe_i) or [144:288]
    # (posR, pe_j), zero elsewhere.
    pos_np = _pos_enc_table()
    posL_np = np.zeros((128, 3 * D_PAIR), dtype=np.float32)
    posR_np = np.zeros((128, 3 * D_PAIR), dtype=np.float32)
    for c in range(3):
        rows = 128 if c < 2 else MAX_LEN - 256
        chunk = pos_np[c * 128 : c * 128 + rows, :]
        posL_np[0:rows, c * D_PAIR : c * D_PAIR + D_HALF] = chunk
        posR_np[0:rows, c * D_PAIR + D_HALF : (c + 1) * D_PAIR] = chunk
    posL_d = nc.inline_tensor(posL_np, "posL_c")
    posR_d = nc.inline_tensor(posR_np, "posR_c")
    iota_np = (
        np.arange(128, dtype=np.float32)[:, None]
        + 128.0 * np.arange(3, dtype=np.float32)[None, :]
    ).astype(np.float32)
    iota_d = nc.inline_tensor(iota_np, "iota")

    with tile.TileContext(nc) as tc, ExitStack() as ctx:
        persist = ctx.enter_context(tc.tile_pool(name="persist", bufs=1))

        # persistent tiles consumed by the j-loop. Matmul tables keep rows at
        # base partitions 0 (j<128) and 64 (j>=128).
        co_t = persist.tile([65, 128 * D_PAIR], dt.bfloat16, tag="co")
        # sepT rows for the per-j sep matmul: sepT[64h, (j%128)*128+i] = sep[i, j]
        sepT_t = persist.tile([65, 128 * IH], dt.bfloat16, tag="sepT")
        ws_t = persist.tile([65, D_PAIR], dt.bfloat16, tag="ws")
        ones_t = persist.tile([65, IH], dt.bfloat16, tag="ones")
        cp_t = persist.tile([IH, D_PAIR], dt.bfloat16, tag="cpt")
        cp2_t = persist.tile([IH, 2 * D_PAIR], dt.bfloat16, tag="cp2t")
        wsepB_t = persist.tile([IH, D_PAIR], dt.bfloat16, tag="wsepB")
        sep_t = persist.tile([IH, L], dt.float32, tag="sept")

        nc.vector.memset(ones_t, 1.0)

        _qs = [nc.sync, nc.scalar]
        _qi = [0]

        def _q():
            _qi[0] ^= 1
            return _qs[_qi[0]]

        with ExitStack() as pre:
            scr = pre.enter_context(tc.tile_pool(name="scr", bufs=1))
            psc = pre.enter_context(tc.tile_pool(name="psc", bufs=1, space="PSUM"))

            # ---- input loads ----
            iota_t = scr.tile([128, 3], dt.float32, tag="iota")
            _q().dma_start(iota_t, iota_d[:, :])

            emb_t = scr.tile([VOCAB, D_HALF], dt.float32, tag="emb")
            _q().dma_start(emb_t, emb_d[:, :])

            w1a = scr.tile([128, D_PAIR], dt.float32, tag="w1a")
            _q().dma_start(w1a, wp_d[0:128, :])
            w1b = scr.tile([16, D_PAIR], dt.float32, tag="w1b")
            _q().dma_start(w1b, wp_d[128:144, :])
            w2a = scr.tile([128, D_PAIR], dt.float32, tag="w2a")
            _q().dma_start(w2a, wp_d[144:272, :])
            w2b = scr.tile([16, D_PAIR], dt.float32, tag="w2b")
            _q().dma_start(w2b, wp_d[272:288, :])

            wsep_f = scr.tile([1, D_PAIR], dt.float32, tag="wsepf")
            _q().dma_start(wsep_f, wp_d[288:289, :])

            bp_t = scr.tile([1, D_PAIR], dt.float32, tag="bp")
            _q().dma_start(bp_t, bp_d[:])

            aab_col = scr.tile([128, 2], dt.int32, tag="aabcol")
            _q().dma_start(
                aab_col,
                bass.AP(tensor=aab_d, offset=0, ap=[[1, 128], [128, 2]]),
            )
            aaB_i = scr.tile([128, L], dt.int32, tag="aaBi")
            _q().dma_start(aaB_i, _bcast(aab_d[:], 128))
            seqB_i = scr.tile([VOCAB, L], dt.int32, tag="seqBi")
            _q().dma_start(seqB_i, _bcast(seqb_d[:], VOCAB))
            seqI_i = scr.tile([VOCAB, IH], dt.int32, tag="seqIi")
            _q().dma_start(seqI_i, _bcast(seqi_d[:], VOCAB))
            aaIB_i = scr.tile([128, IH], dt.int32, tag="aaIBi")
            _q().dma_start(aaIB_i, _bcast(aai_d[:], 128))

            posL = scr.tile([128, 3 * D_PAIR], dt.float32, tag="posL")
            _q().dma_start(posL, posL_d[:, :])
            posR = scr.tile([128, 3 * D_PAIR], dt.float32, tag="posR")
            _q().dma_start(posR, posR_d[:, :])

            # ---- int -> f32 casts ----
            aab_colf = scr.tile([128, 2], dt.float32, tag="aabcolf")
            nc.vector.tensor_copy(aab_colf, aab_col)
            aaB_f = scr.tile([128, L], dt.float32, tag="aaBf")
            nc.vector.tensor_copy(aaB_f, aaB_i)
            seqB_f = scr.tile([VOCAB, L], dt.float32, tag="seqBf")
            nc.vector.tensor_copy(seqB_f, seqB_i)
            seqI_f = scr.tile([VOCAB, IH], dt.float32, tag="seqIf")
            nc.vector.tensor_copy(seqI_f, seqI_i)
            aaIB_f = scr.tile([128, IH], dt.float32, tag="aaIBf")
            nc.vector.tensor_copy(aaIB_f, aaIB_i)

            # ---- one-hots ----
            ohSeq = scr.tile([VOCAB, L], dt.float32, tag="ohSeq")
            nc.vector.tensor_scalar(
                ohSeq, seqB_f, iota_t[0:VOCAB, 0:1], None, ALU.is_equal
            )
            ohSeqI = scr.tile([VOCAB, IH], dt.float32, tag="ohSeqI")
            nc.vector.tensor_scalar(
                ohSeqI, seqI_f, iota_t[0:VOCAB, 0:1], None, ALU.is_equal
            )
            ohP = []
            ohPi = []
            for c in range(3):
                t = scr.tile([128, L], dt.float32, tag=f"ohP{c}", name=f"ohP{c}")
                nc.vector.tensor_scalar(t, aaB_f, iota_t[:, c : c + 1], None, ALU.is_equal)
                ohP.append(t)
                ti = scr.tile([128, IH], dt.float32, tag=f"ohPi{c}", name=f"ohPi{c}")
                nc.vector.tensor_scalar(
                    ti, aaIB_f, iota_t[:, c : c + 1], None, ALU.is_equal
                )
                ohPi.append(ti)

            # ---- seT = emb^T gathered by seq: (144, L) split 128+16 rows ----
            seT_a_ps = psc.tile([128, L], dt.float32, tag="seTaP")
            nc.tensor.matmul(seT_a_ps, emb_t[:, 0:128], ohSeq, start=True, stop=True)
            seT_b_ps = psc.tile([16, L], dt.float32, tag="seTbP")
            nc.tensor.matmul(
                seT_b_ps, emb_t[:, 128:D_HALF], ohSeq, start=True, stop=True
            )
            seT_a = scr.tile([128, L], dt.float32, tag="seTa")
            nc.vector.tensor_copy(seT_a, seT_a_ps)
            seT_b = scr.tile([16, L], dt.float32, tag="seTb")
            nc.vector.tensor_copy(seT_b, seT_b_ps)

            seTi_a_ps = psc.tile([128, IH], dt.float32, tag="seTiaP")
            nc.tensor.matmul(
                seTi_a_ps, emb_t[:, 0:128], ohSeqI, start=True, stop=True
            )
            seTi_b_ps = psc.tile([16, IH], dt.float32, tag="seTibP")
            nc.tensor.matmul(
                seTi_b_ps, emb_t[:, 128:D_HALF], ohSeqI, start=True, stop=True
            )
            seTi_a = scr.tile([128, IH], dt.float32, tag="seTia")
            nc.vector.tensor_copy(seTi_a, seTi_a_ps)
            seTi_b = scr.tile([16, IH], dt.float32, tag="seTib")
            nc.vector.tensor_copy(seTi_b, seTi_b_ps)

            # ---- Co halves -> bf16 -> flat rows at partitions 0 / 64 ----
            for h in range(2):
                co_ps = psc.tile(
                    [128, D_PAIR], dt.float32, tag=f"co{h}", name=f"co{h}"
                )
                sl = slice(h * 128, (h + 1) * 128)
                nc.tensor.matmul(co_ps, seT_a[:, sl], w1a, start=True, stop=False)
                nc.tensor.matmul(co_ps, seT_b[:, sl], w1b, start=False, stop=False)
                for c in range(3):
                    nc.tensor.matmul(
                        co_ps,
                        ohP[c][:, sl],
                        posR[:, c * D_PAIR : (c + 1) * D_PAIR],
                        start=False,
                        stop=(c == 2),
                    )
                co_bf = scr.tile(
                    [128, D_PAIR], dt.bfloat16, tag=f"cobf{h}", name=f"cobf{h}"
                )
                nc.vector.tensor_copy(co_bf, co_ps)
                _q().dma_start(co_t[64 * h : 64 * h + 1, :], co_bf)

            # ---- Cp = se_i @ W2 + b_proj + [pe_i | 0], bf16 ----
            ones_f = scr.tile([1, IH], dt.float32, tag="onesf")
            nc.vector.memset(ones_f, 1.0)
            cp_ps = psc.tile([128, D_PAIR], dt.float32, tag="cpP")
            nc.tensor.matmul(cp_ps, seTi_a, w2a, start=True, stop=False)
            nc.tensor.matmul(cp_ps, seTi_b, w2b, start=False, stop=False)
            for c in range(3):
                nc.tensor.matmul(
                    cp_ps,
                    ohPi[c],
                    posL[:, c * D_PAIR : (c + 1) * D_PAIR],
                    start=False,
                    stop=False,
                )
            nc.tensor.matmul(cp_ps, ones_f, bp_t, start=False, stop=True)
            nc.vector.tensor_copy(cp_t, cp_ps)
            nc.vector.tensor_copy(cp2_t[:, 0:D_PAIR], cp_t)
            nc.vector.tensor_copy(cp2_t[:, D_PAIR : 2 * D_PAIR], cp_t)

            # ---- sepT_h[jj, i] = ln(|aa_i - aa_j|+1) -> S8 batched rows ----
            for h in range(2):
                dist_t = scr.tile([128, IH], dt.float32, tag=f"dist{h}", name=f"dist{h}")
                nc.vector.tensor_scalar(
                    dist_t, aaIB_f, aab_colf[:, h : h + 1], None, ALU.subtract
                )
                abs_t = scr.tile([128, IH], dt.float32, tag=f"abs{h}", name=f"abs{h}")
                nc.scalar.activation(abs_t, dist_t, AF.Abs)
                sep_f = scr.tile([128, IH], dt.float32, tag=f"sepf{h}", name=f"sepf{h}")
                nc.scalar.activation(sep_f, abs_t, AF.Ln, bias=1.0)
                sep_bf = scr.tile(
                    [128, IH], dt.bfloat16, tag=f"sepbf{h}", name=f"sepbf{h}"
                )
                nc.vector.tensor_copy(sep_bf, sep_f)
                _q().dma_start(sepT_t[64 * h : 64 * h + 1, :], sep_bf)

            # ---- Z selector from wsep ----
            wsep_bf = scr.tile([1, D_PAIR], dt.bfloat16, tag="wsepbf")
            nc.vector.tensor_copy(wsep_bf, wsep_f)
            _q().dma_start(ws_t[0:1, :], wsep_bf)
            _q().dma_start(ws_t[64:65, :], wsep_bf)

            # ---- wsep broadcast rows + sep in i-partition layout (V/W) ----
            wsepF_t = scr.tile([IH, D_PAIR], dt.float32, tag="wsepF")
            _q().dma_start(wsepF_t, _bcast(wp_d[288:289, :], 128))
            nc.vector.tensor_copy(wsepB_t, wsepF_t)
            aaCol_i = scr.tile([IH, 1], dt.int32, tag="aaColi")
            _q().dma_start(aaCol_i, aai_d[:])
            aaCol_f = scr.tile([IH, 1], dt.float32, tag="aaColf")
            nc.vector.tensor_copy(aaCol_f, aaCol_i)
            distI_t = scr.tile([IH, L], dt.float32, tag="distI")
            nc.vector.tensor_scalar(distI_t, aaB_f, aaCol_f, None, ALU.subtract)
            absI_t = scr.tile([IH, L], dt.float32, tag="absI")
            nc.scalar.activation(absI_t, distI_t, AF.Abs)
            nc.scalar.activation(sep_t, absI_t, AF.Ln, bias=1.0)

        if stage == "setup":
            dbg = ctx.enter_context(tc.tile_pool(name="dbg", bufs=1))
            dbf = dbg.tile([IH, D_PAIR], dt.bfloat16, tag="dbf")
            nc.vector.tensor_copy(dbf, cp_t)
            nc.sync.dma_start(out_d[:, 0:1, :], dbf)
            return nc

        # ---- j loop ----
        psj = ctx.enter_context(tc.tile_pool(name="psj", bufs=8, space="PSUM"))
        obp = ctx.enter_context(tc.tile_pool(name="obp", bufs=obufs))
        ngroups = int(stage[5:]) if stage.startswith("jloop") else L // jg
        if variant == "dmaonly":
            obs = []
            for k in range(2):
                t = obp.tile([IH, jg * D_PAIR], dt.bfloat16, tag="ob", name="ob")
                nc.vector.memset(t, 0.5)
                obs.append(t)
            for g in range(ngroups * repeat):
                g = g % ngroups
                eng = nc.sync if g % 2 == 0 else nc.scalar
                eng.dma_start(out_d[:, g * jg : (g + 1) * jg, :], obs[g % 2])
            return nc
        vset = set(variant.split("+")) if variant else set()
        seq = _pattern_seq(pat)
        if "nopair" not in vset:
            # block of 16 with secondary (cp-add) ops in adjacent pairs so one
            # 576-wide gps/DVE op covers two j's (halves dispatch overhead)
            blk = ["P", "V", "V", "Q", "V", "V", "P", "W",
                   "Q", "V", "V", "P", "Q", "W", "P", "Q"]
            seq = blk * (L // 16)
        if "allP" in vset:
            seq = ["P"] * L
        SUB = 4 if "sub4" in vset else 8
        for g in range(ngroups * repeat):
            g = g % ngroups
            ob = obp.tile([IH, jg * D_PAIR], dt.bfloat16, tag="ob", name="ob")
            for s in range(jg // SUB):
                js = [g * jg + s * SUB + t for t in range(SUB)]
                pss = []
                for j in js:
                    b, jo = 64 * (j // 128), j % 128
                    code = seq[j]
                    ps = psj.tile([IH, D_PAIR], dt.float32, tag="ps", name="ps")
                    pe_sep = code in ("P", "Q", "B") and "mm1" not in vset
                    nc.tensor.matmul(
                        ps,
                        ones_t[b : b + 1, :],
                        co_t[b : b + 1, jo * D_PAIR : (jo + 1) * D_PAIR],
                        start=True,
                        stop=not pe_sep,
                    )
                    pss.append(ps)
                for ps, j in zip(pss, js):
                    code = seq[j]
                    if code in ("P", "Q", "B") and "mm1" not in vset:
                        b, jo = 64 * (j // 128), j % 128
                        nc.tensor.matmul(
                            ps,
                            sepT_t[b : b + 1, jo * IH : (jo + 1) * IH],
                            ws_t[b : b + 1, :],
                            start=False,
                            stop=True,
                        )
                pend_v = []
                for ps, j in zip(pss, js):
                    jj = j - g * jg
                    osl = ob[:, jj * D_PAIR : (jj + 1) * D_PAIR]
                    code = seq[j]
                    if "nodrain" in vset:
                        continue
                    if "actonly" in vset:
                        nc.scalar.activation(osl, ps, AF.Identity)
                        continue
                    if code == "P":
                        nc.vector.tensor_tensor(osl, ps, cp_t, ALU.add)
                    elif code == "Q":
                        nc.scalar.activation(osl, ps, AF.Identity)
                        nc.gpsimd.tensor_tensor(osl, osl, cp_t, ALU.add)
                    elif code == "V":
                        nc.vector.scalar_tensor_tensor(
                            osl, wsepB_t, sep_t[:, j : j + 1], ps,
                            ALU.mult, ALU.add,
                        )
                        pend_v.append(jj)
                        if len(pend_v) == 2 and pend_v[1] == pend_v[0] + 1:
                            j0 = pend_v[0]
                            osl2 = ob[:, j0 * D_PAIR : (j0 + 2) * D_PAIR]
                            nc.gpsimd.tensor_tensor(osl2, osl2, cp2_t, ALU.add)
                            pend_v = []
                        elif len(pend_v) == 2:
                            for jx in pend_v:
                                oslx = ob[:, jx * D_PAIR : (jx + 1) * D_PAIR]
                                nc.gpsimd.tensor_tensor(oslx, oslx, cp_t, ALU.add)
                            pend_v = []
                    else:  # W
                        nc.vector.scalar_tensor_tensor(
                            osl, wsepB_t, sep_t[:, j : j + 1], ps, ALU.mult, ALU.add
                        )
                        nc.vector.tensor_tensor(osl, osl, cp_t, ALU.add)
                for jx in pend_v:
                    oslx = ob[:, jx * D_PAIR : (jx + 1) * D_PAIR]
                    nc.gpsimd.tensor_tensor(oslx, oslx, cp_t, ALU.add)
            if "nodma" not in vset and "nodrain" not in vset:
                eng = nc.sync if (g % 2 == 0 or "allsync" in vset) else nc.scalar
                eng.dma_start(out_d[:, g * jg : (g + 1) * jg, :], ob)

    return nc


_NC_CACHE = []


def make_in_maps(seq, aa_idx, emb_table, W_proj, b_proj):
    seq = np.asarray(seq, dtype=np.int32)
    aa_idx = np.asarray(aa_idx, dtype=np.int32)
    emb_table = np.ascontiguousarray(np.asarray(emb_table, dtype=np.float32))
    W_proj = np.ascontiguousarray(np.asarray(W_proj, dtype=np.float32))
    b_proj = np.ascontiguousarray(np.asarray(b_proj, dtype=np.float32))
    in_maps = []
    for c in range(N_CORES):
        b, ih = c // 2, c % 2
        in_maps.append(
            {
                "seqb": np.ascontiguousarray(seq[b]),
                "seqi": np.ascontiguousarray(seq[b, ih * IH : (ih + 1) * IH]),
                "aab": np.ascontiguousarray(aa_idx[b]),
                "aai": np.ascontiguousarray(aa_idx[b, ih * IH : (ih + 1) * IH]),
                "emb": emb_table,
                "wp": W_proj,
                "bp": b_proj,
            }
        )
    return in_maps


def gather_out(results) -> np.ndarray:
    out = np.empty((B, L, L, D_PAIR), dtype=np.float32)
    for c in range(N_CORES):
        b, ih = c // 2, c % 2
        out[b, ih * IH : (ih + 1) * IH] = np.asarray(results[c]["out"]).astype(
            np.float32
        )
    return out


def kernel(seq, aa_idx, emb_table, W_proj, b_proj) -> np.ndarray:
    if not _NC_CACHE:
        nc = build()
        nc.finalize()
        _NC_CACHE.append(nc)
    nc = _NC_CACHE[0]
    in_maps = make_in_maps(seq, aa_idx, emb_table, W_proj, b_proj)
    res = run_bass_kernel_spmd(nc, in_maps, core_ids=list(range(N_CORES)))
    return gather_out(res.results)



# revision 21
# speedup vs baseline: 1.2274x; 1.2274x over previous
"""PairEmbedding Bass kernel for 8 TRN2 NeuronCores.

out[b,i,j,:] = Co[b,j,:] + Cp[b,i,:] + sep(b,i,j) * w_sep
  Co[j] = se_j @ W1 + [0 | pe_j]
  Cp[i] = se_i @ W2 + b_proj + [pe_i | 0]
  sep(i,j) = ln(|aa_i - aa_j| + 1)
where se = emb_table[seq], pe = pos_table[aa_idx], W1 = W_proj[0:144],
W2 = W_proj[144:288], w_sep = W_proj[288].

Sharding: core c -> batch b = c//2, row block i in [128*(c%2), 128*(c%2)+128),
all 256 j. Per-core output (128, 256, 288) stored bf16, host upcasts to f32
(bf16 rounding ~0.4% << the 2e-2 gate).

j-loop design (v1, cost-model-driven):
  Per j ONE K=2 matmul into PSUM: psum[i,d] = 1*co[j,d] + sep[i,j]*w[d]
  (stationary = [ones_row; sepT_row_j], moving = [co_row_j; w]).
  Drains are QUAD-batched (4 j's per instruction, strided read across 4
  half-banks of one PSUM tile) to amortize PSUM access latency:
    P: DVE   ob4 = ps4 + cp4                       (f32 psum read)
    Q: ACT   ob4 = Iden(ps4); DVE ob4 += cp4       (bf16 2x mode add)
    G: ACT   ob4 = Iden(ps4); GPS ob4 += cp4
    D: GPS   ob4 = ps4 + cp4                       (gpsimd psum read)
  DMA: groups of JG=32 j's (2.36 MB) alternating sync/scalar HWDGE queues.
Matmul operands indexed along the free dim from base partitions {0, 64}
(HW requires base partition in {0,32,64}); K=2 rows sit on partitions
{b, b+1}.
"""

import math
from contextlib import ExitStack

import numpy as np

from concourse import bacc, bass, mybir, tile
from concourse.bass_utils import run_bass_kernel_spmd

dt = mybir.dt
AF = mybir.ActivationFunctionType
ALU = mybir.AluOpType

B = 4
L = 256
D_PAIR = 288
D_HALF = 144
MAX_LEN = 260
VOCAB = 21
IH = 128          # i rows per core
JG = 32           # j's per output DMA group
N_CORES = 8
PSB = 512         # f32 slots per PSUM bank


def _pos_enc_table() -> np.ndarray:
    idx = np.arange(0, D_HALF, 2, dtype=np.float32)
    t = (np.float32(math.log(10000.0)) * idx) / np.float32(D_HALF)
    denom = np.exp(t, dtype=np.float32)
    pos = np.arange(MAX_LEN, dtype=np.float32)[:, None]
    pe = np.zeros((MAX_LEN, D_HALF), dtype=np.float32)
    pe[:, 0::2] = np.sin(pos / denom, dtype=np.float32)
    pe[:, 1::2] = np.cos(pos / denom, dtype=np.float32)
    return pe


def _bcast(ap_src, nparts: int):
    return bass.AP(
        tensor=ap_src.tensor, offset=ap_src.offset, ap=[[0, nparts], *ap_src.ap]
    )


def _quad_seq(pat: str, nquads: int):
    """'P:18,Q:30,G:6,D:10' (batch counts) -> interleaved length-nquads codes.
    'seq:PQDQ' -> the literal cycle repeated to length nquads."""
    if pat.startswith("seq:"):
        cyc = pat[4:]
        assert nquads % len(cyc) == 0, (cyc, nquads)
        return list(cyc) * (nquads // len(cyc))
    counts = {}
    for part in pat.split(","):
        k, v = part.split(":")
        counts[k.strip()] = int(v)
    assert sum(counts.values()) == nquads, (counts, nquads)
    seq = []
    used = {k: 0 for k in counts}
    for i in range(1, nquads + 1):
        k = max(counts, key=lambda c: counts[c] * i / nquads - used[c])
        seq.append(k)
        used[k] += 1
    return seq


def build(
    stage: str = "full",
    repeat: int = 1,
    variant: str = "",
    jg: int = 16,
    pat: str = "cell:QGQ,GQP,QGQ,GQP",
    obufs: int = 4,
    db: int = 4,
    psbufs: int = 0,
) -> bass.Bass:
    psbufs = psbufs or (4 if pat.startswith("cell:") else 8 // db)
    nc = bacc.Bacc("TRN2", target_bir_lowering=False)

    seqb_d = nc.dram_tensor("seqb", [L], dt.int32, kind="ExternalInput")
    seqi_d = nc.dram_tensor("seqi", [IH], dt.int32, kind="ExternalInput")
    aab_d = nc.dram_tensor("aab", [L], dt.int32, kind="ExternalInput")
    aai_d = nc.dram_tensor("aai", [IH], dt.int32, kind="ExternalInput")
    emb_d = nc.dram_tensor("emb", [VOCAB, D_HALF], dt.float32, kind="ExternalInput")
    wp_d = nc.dram_tensor("wp", [D_PAIR + 1, D_PAIR], dt.float32, kind="ExternalInput")
    bp_d = nc.dram_tensor("bp", [D_PAIR], dt.float32, kind="ExternalInput")
    out_d = nc.dram_tensor("out", [IH, L, D_PAIR], dt.bfloat16, kind="ExternalOutput")

    # pos-table gather sources, pre-arranged on host: chunk c of <=128 pos
    # rows on partitions, channel slice [0:144] (posL, pe_i) or [144:288]
    # (posR, pe_j), zero elsewhere.
    pos_np = _pos_enc_table()
    posL_np = np.zeros((128, 3 * D_PAIR), dtype=np.float32)
    posR_np = np.zeros((128, 3 * D_PAIR), dtype=np.float32)
    for c in range(3):
        rows = 128 if c < 2 else MAX_LEN - 256
        chunk = pos_np[c * 128 : c * 128 + rows, :]
        posL_np[0:rows, c * D_PAIR : c * D_PAIR + D_HALF] = chunk
        posR_np[0:rows, c * D_PAIR + D_HALF : (c + 1) * D_PAIR] = chunk
    posL_d = nc.inline_tensor(posL_np, "posL_c")
    posR_d = nc.inline_tensor(posR_np, "posR_c")
    iota_np = (
        np.arange(128, dtype=np.float32)[:, None]
        + 128.0 * np.arange(3, dtype=np.float32)[None, :]
    ).astype(np.float32)
    iota_d = nc.inline_tensor(iota_np, "iota")

    with tile.TileContext(nc) as tc, ExitStack() as ctx:
        persist = ctx.enter_context(tc.tile_pool(name="persist", bufs=1))

        # persistent tiles consumed by the j-loop.
        # st2: K=2 stationary rows. partition b   = ones (j block b),
        #                           partition b+1 = sepT rows: [jo*128+i] = sep[i, jo]
        st2_t = persist.tile([66, 128 * IH], dt.bfloat16, tag="st2")
        # cw:  K=2 moving rows.     partition b   = co rows: [jo*288+d] = co[jo, d]
        #                           partition b+1 = w_sep repeated per j
        cw_t = persist.tile([66, 128 * D_PAIR], dt.bfloat16, tag="cw")
        cp_t = persist.tile([IH, D_PAIR], dt.bfloat16, tag="cpt")
        cp4_t = persist.tile([IH, 4 * D_PAIR], dt.bfloat16, tag="cp4t")
        cell = pat.startswith("cell:")
        if cell:
            # coB2[i, s*288+d] = co[4s+3, d] + cp[i, d]: whole-column source
            # for PE-free stt j's (j % 4 == 3)
            coB2_t = persist.tile([IH, (L // 4) * D_PAIR], dt.bfloat16, tag="coB2")
            wsepB_t = persist.tile([IH, D_PAIR], dt.bfloat16, tag="wsepB")
            sep_t = persist.tile([IH, L], dt.float32, tag="sept")

        _qs = [nc.sync, nc.scalar]
        _qi = [0]

        def _q():
            _qi[0] ^= 1
            return _qs[_qi[0]]

        with ExitStack() as pre:
            scr = pre.enter_context(tc.tile_pool(name="scr", bufs=1))
            psc = pre.enter_context(tc.tile_pool(name="psc", bufs=1, space="PSUM"))

            # ---- input loads ----
            iota_t = scr.tile([128, 3], dt.float32, tag="iota")
            _q().dma_start(iota_t, iota_d[:, :])

            emb_t = scr.tile([VOCAB, D_HALF], dt.float32, tag="emb")
            _q().dma_start(emb_t, emb_d[:, :])

            w1a = scr.tile([128, D_PAIR], dt.float32, tag="w1a")
            _q().dma_start(w1a, wp_d[0:128, :])
            w1b = scr.tile([16, D_PAIR], dt.float32, tag="w1b")
            _q().dma_start(w1b, wp_d[128:144, :])
            w2a = scr.tile([128, D_PAIR], dt.float32, tag="w2a")
            _q().dma_start(w2a, wp_d[144:272, :])
            w2b = scr.tile([16, D_PAIR], dt.float32, tag="w2b")
            _q().dma_start(w2b, wp_d[272:288, :])

            wsep_f = scr.tile([1, D_PAIR], dt.float32, tag="wsepf")
            _q().dma_start(wsep_f, wp_d[288:289, :])

            bp_t = scr.tile([1, D_PAIR], dt.float32, tag="bp")
            _q().dma_start(bp_t, bp_d[:])

            aab_col = scr.tile([128, 2], dt.int32, tag="aabcol")
            _q().dma_start(
                aab_col,
                bass.AP(tensor=aab_d, offset=0, ap=[[1, 128], [128, 2]]),
            )
            aaB_i = scr.tile([128, L], dt.int32, tag="aaBi")
            _q().dma_start(aaB_i, _bcast(aab_d[:], 128))
            seqB_i = scr.tile([VOCAB, L], dt.int32, tag="seqBi")
            _q().dma_start(seqB_i, _bcast(seqb_d[:], VOCAB))
            seqI_i = scr.tile([VOCAB, IH], dt.int32, tag="seqIi")
            _q().dma_start(seqI_i, _bcast(seqi_d[:], VOCAB))
            aaIB_i = scr.tile([128, IH], dt.int32, tag="aaIBi")
            _q().dma_start(aaIB_i, _bcast(aai_d[:], 128))

            posL = scr.tile([128, 3 * D_PAIR], dt.float32, tag="posL")
            _q().dma_start(posL, posL_d[:, :])
            posR = scr.tile([128, 3 * D_PAIR], dt.float32, tag="posR")
            _q().dma_start(posR, posR_d[:, :])

            # ---- int -> f32 casts ----
            aab_colf = scr.tile([128, 2], dt.float32, tag="aabcolf")
            nc.vector.tensor_copy(aab_colf, aab_col)
            aaB_f = scr.tile([128, L], dt.float32, tag="aaBf")
            nc.vector.tensor_copy(aaB_f, aaB_i)
            seqB_f = scr.tile([VOCAB, L], dt.float32, tag="seqBf")
            nc.vector.tensor_copy(seqB_f, seqB_i)
            seqI_f = scr.tile([VOCAB, IH], dt.float32, tag="seqIf")
            nc.vector.tensor_copy(seqI_f, seqI_i)
            aaIB_f = scr.tile([128, IH], dt.float32, tag="aaIBf")
            nc.vector.tensor_copy(aaIB_f, aaIB_i)

            # ---- one-hots ----
            ohSeq = scr.tile([VOCAB, L], dt.float32, tag="ohSeq")
            nc.vector.tensor_scalar(
                ohSeq, seqB_f, iota_t[0:VOCAB, 0:1], None, ALU.is_equal
            )
            ohSeqI = scr.tile([VOCAB, IH], dt.float32, tag="ohSeqI")
            nc.vector.tensor_scalar(
                ohSeqI, seqI_f, iota_t[0:VOCAB, 0:1], None, ALU.is_equal
            )
            ohP = []
            ohPi = []
            for c in range(3):
                t = scr.tile([128, L], dt.float32, tag=f"ohP{c}", name=f"ohP{c}")
                nc.vector.tensor_scalar(t, aaB_f, iota_t[:, c : c + 1], None, ALU.is_equal)
                ohP.append(t)
                ti = scr.tile([128, IH], dt.float32, tag=f"ohPi{c}", name=f"ohPi{c}")
                nc.vector.tensor_scalar(
                    ti, aaIB_f, iota_t[:, c : c + 1], None, ALU.is_equal
                )
                ohPi.append(ti)

            # ---- seT = emb^T gathered by seq: (144, L) split 128+16 rows ----
            seT_a_ps = psc.tile([128, L], dt.float32, tag="seTaP")
            nc.tensor.matmul(seT_a_ps, emb_t[:, 0:128], ohSeq, start=True, stop=True)
            seT_b_ps = psc.tile([16, L], dt.float32, tag="seTbP")
            nc.tensor.matmul(
                seT_b_ps, emb_t[:, 128:D_HALF], ohSeq, start=True, stop=True
            )
            seT_a = scr.tile([128, L], dt.float32, tag="seTa")
            nc.vector.tensor_copy(seT_a, seT_a_ps)
            seT_b = scr.tile([16, L], dt.float32, tag="seTb")
            nc.vector.tensor_copy(seT_b, seT_b_ps)

            seTi_a_ps = psc.tile([128, IH], dt.float32, tag="seTiaP")
            nc.tensor.matmul(
                seTi_a_ps, emb_t[:, 0:128], ohSeqI, start=True, stop=True
            )
            seTi_b_ps = psc.tile([16, IH], dt.float32, tag="seTibP")
            nc.tensor.matmul(
                seTi_b_ps, emb_t[:, 128:D_HALF], ohSeqI, start=True, stop=True
            )
            seTi_a = scr.tile([128, IH], dt.float32, tag="seTia")
            nc.vector.tensor_copy(seTi_a, seTi_a_ps)
            seTi_b = scr.tile([16, IH], dt.float32, tag="seTib")
            nc.vector.tensor_copy(seTi_b, seTi_b_ps)

            # ---- Co halves -> bf16 -> flat rows at partitions 0 / 64 ----
            for h in range(2):
                co_ps = psc.tile(
                    [128, D_PAIR], dt.float32, tag=f"co{h}", name=f"co{h}"
                )
                sl = slice(h * 128, (h + 1) * 128)
                nc.tensor.matmul(co_ps, seT_a[:, sl], w1a, start=True, stop=False)
                nc.tensor.matmul(co_ps, seT_b[:, sl], w1b, start=False, stop=False)
                for c in range(3):
                    nc.tensor.matmul(
                        co_ps,
                        ohP[c][:, sl],
                        posR[:, c * D_PAIR : (c + 1) * D_PAIR],
                        start=False,
                        stop=(c == 2),
                    )
                co_bf = scr.tile(
                    [128, D_PAIR], dt.bfloat16, tag=f"cobf{h}", name=f"cobf{h}"
                )
                nc.vector.tensor_copy(co_bf, co_ps)
                _q().dma_start(cw_t[64 * h : 64 * h + 1, :], co_bf)

            # ---- Cp = se_i @ W2 + b_proj + [pe_i | 0], bf16 ----
            ones_f = scr.tile([1, IH], dt.float32, tag="onesf")
            nc.vector.memset(ones_f, 1.0)
            cp_ps = psc.tile([128, D_PAIR], dt.float32, tag="cpP")
            nc.tensor.matmul(cp_ps, seTi_a, w2a, start=True, stop=False)
            nc.tensor.matmul(cp_ps, seTi_b, w2b, start=False, stop=False)
            for c in range(3):
                nc.tensor.matmul(
                    cp_ps,
                    ohPi[c],
                    posL[:, c * D_PAIR : (c + 1) * D_PAIR],
                    start=False,
                    stop=False,
                )
            nc.tensor.matmul(cp_ps, ones_f, bp_t, start=False, stop=True)
            nc.vector.tensor_copy(cp_t, cp_ps)
            for r in range(4):
                nc.vector.tensor_copy(
                    cp4_t[:, r * D_PAIR : (r + 1) * D_PAIR], cp_t
                )

            # ---- sepT rows -> st2 partitions 1 / 65 ----
            for h in range(2):
                dist_t = scr.tile([128, IH], dt.float32, tag=f"dist{h}", name=f"dist{h}")
                nc.vector.tensor_scalar(
                    dist_t, aaIB_f, aab_colf[:, h : h + 1], None, ALU.subtract
                )
                abs_t = scr.tile([128, IH], dt.float32, tag=f"abs{h}", name=f"abs{h}")
                nc.scalar.activation(abs_t, dist_t, AF.Abs)
                sep_f = scr.tile([128, IH], dt.float32, tag=f"sepf{h}", name=f"sepf{h}")
                nc.scalar.activation(sep_f, abs_t, AF.Ln, bias=1.0)
                sep_bf = scr.tile(
                    [128, IH], dt.bfloat16, tag=f"sepbf{h}", name=f"sepbf{h}"
                )
                nc.vector.tensor_copy(sep_bf, sep_f)
                _q().dma_start(st2_t[64 * h + 1 : 64 * h + 2, :], sep_bf)

            # ---- ones rows -> st2 partitions 0 / 64 ----
            ones128 = scr.tile([128, IH], dt.bfloat16, tag="ones128")
            nc.vector.memset(ones128, 1.0)
            _q().dma_start(st2_t[0:1, :], ones128)
            _q().dma_start(st2_t[64:65, :], ones128)

            # ---- w_sep repeated rows -> cw partitions 1 / 65 ----
            wsep_bf = scr.tile([1, D_PAIR], dt.bfloat16, tag="wsepbf")
            nc.vector.tensor_copy(wsep_bf, wsep_f)
            wsrep_src = bass.AP(
                tensor=wsep_bf.tensor,
                offset=wsep_bf.offset,
                ap=[wsep_bf.ap[0], [0, 128], [1, D_PAIR]],
            )
            _q().dma_start(cw_t[1:2, :], wsrep_src)
            _q().dma_start(cw_t[65:66, :], wsrep_src)

            if cell:
                # ---- wsep broadcast rows + sep in i-partition layout ----
                wsepF_t = scr.tile([IH, D_PAIR], dt.float32, tag="wsepF")
                _q().dma_start(wsepF_t, _bcast(wp_d[288:289, :], 128))
                nc.vector.tensor_copy(wsepB_t, wsepF_t)
                aaCol_i = scr.tile([IH, 1], dt.int32, tag="aaColi")
                _q().dma_start(aaCol_i, aai_d[:])
                aaCol_f = scr.tile([IH, 1], dt.float32, tag="aaColf")
                nc.vector.tensor_copy(aaCol_f, aaCol_i)
                distI_t = scr.tile([IH, L], dt.float32, tag="distI")
                nc.vector.tensor_scalar(distI_t, aaB_f, aaCol_f, None, ALU.subtract)
                absI_t = scr.tile([IH, L], dt.float32, tag="absI")
                nc.scalar.activation(absI_t, distI_t, AF.Abs)
                nc.scalar.activation(sep_t, absI_t, AF.Ln, bias=1.0)

        if cell:
            # ---- coB2 = ones (x) co_row + cp, one slot per stt-j ----
            # stt j's are j % 8 in {6, 7}; slot s = (j//8)*2 + (j%8 - 6)
            with ExitStack() as pre2:
                ps2 = pre2.enter_context(
                    tc.tile_pool(name="ps2", bufs=4, space="PSUM")
                )
                for s in range(L // 4):
                    j = 8 * (s // 2) + 6 + (s % 2)
                    b, jo = 64 * (j // 128), j % 128
                    px = ps2.tile([IH, D_PAIR], dt.float32, tag="cob", name="cob")
                    nc.tensor.matmul(
                        px,
                        st2_t[b : b + 1, jo * IH : (jo + 1) * IH],
                        cw_t[b : b + 1, jo * D_PAIR : (jo + 1) * D_PAIR],
                        start=True,
                        stop=True,
                    )
                    nc.vector.tensor_tensor(
                        coB2_t[:, s * D_PAIR : (s + 1) * D_PAIR], px, cp_t, ALU.add
                    )

        if stage == "setup":
            dbg = ctx.enter_context(tc.tile_pool(name="dbg", bufs=1))
            dbf = dbg.tile([IH, D_PAIR], dt.bfloat16, tag="dbf")
            nc.vector.tensor_copy(dbf, cp_t)
            nc.sync.dma_start(out_d[:, 0:1, :], dbf)
            return nc

        # ---- j loop ----
        psj = ctx.enter_context(tc.tile_pool(name="psj", bufs=psbufs, space="PSUM"))
        obp = ctx.enter_context(tc.tile_pool(name="obp", bufs=obufs))
        ngroups = int(stage[5:]) if stage.startswith("jloop") else L // jg
        if variant == "dmaonly":
            obs = []
            for k in range(2):
                t = obp.tile([IH, jg * D_PAIR], dt.bfloat16, tag="ob", name="ob")
                nc.vector.memset(t, 0.5)
                obs.append(t)
            for g in range(ngroups * repeat):
                g = g % ngroups
                eng = nc.sync if g % 2 == 0 else nc.scalar
                eng.dma_start(out_d[:, g * jg : (g + 1) * jg, :], obs[g % 2])
            return nc
        vset = set(variant.split("+")) if variant else set()
        if cell:
            cyc = pat[5:].split(",")
            ncells_g = jg // 8
            psj2 = ctx.enter_context(tc.tile_pool(name="psj2", bufs=4, space="PSUM"))

            def drain(code, psap, obsl, jlist, pend, w):
                cpb = cp4_t[:, 0 : w * D_PAIR]
                if code == "P":
                    nc.vector.tensor_tensor(obsl, psap, cpb, ALU.add)
                elif code == "Q":
                    nc.scalar.activation(obsl, psap, AF.Identity)
                    pend["dve"].extend(jlist)
                elif code == "G":
                    nc.scalar.activation(obsl, psap, AF.Identity)
                    pend["gps"].extend(jlist)
                elif code == "E":
                    nc.gpsimd.tensor_copy(obsl, psap)
                    pend["dve"].extend(jlist)
                elif code == "F":
                    nc.gpsimd.tensor_copy(obsl, psap)
                    pend["gps"].extend(jlist)
                else:  # D
                    nc.gpsimd.tensor_tensor(obsl, psap, cpb, ALU.add)

            def flush(pend, ob):
                for eng_name, lst in pend.items():
                    eng = nc.vector if eng_name == "dve" else nc.gpsimd
                    lst.sort()
                    run = []
                    for jj in lst + [None]:
                        if run and (
                            jj is None or jj != run[-1] + 1 or len(run) == 4
                        ):
                            w = len(run)
                            sl = ob[:, run[0] * D_PAIR : (run[0] + w) * D_PAIR]
                            eng.tensor_tensor(
                                sl, sl, cp4_t[:, 0 : w * D_PAIR], ALU.add
                            )
                            run = []
                        if jj is not None:
                            run.append(jj)
                    lst.clear()

            skew = 3  # pairs of drain lag behind matmul emission
            for g in range(ngroups * repeat):
                g = g % ngroups
                ob = obp.tile([IH, jg * D_PAIR], dt.bfloat16, tag="ob", name="ob")
                pend = {"dve": [], "gps": []}
                # flat pair list for the group: (code, j, tile placeholder)
                plan = []
                for cidx in range(ncells_g):
                    j0 = g * jg + cidx * 8
                    codes = cyc[((j0 % L) // 8) % len(cyc)]
                    for pidx in range(3):
                        plan.append((codes[pidx], j0 + 2 * pidx))
                stts = [g * jg + 8 * c + 6 + r for c in range(ncells_g) for r in range(2)]
                tiles = [None] * len(plan)

                def emit_drain(idx):
                    code, jp = plan[idx]
                    pt2 = tiles[idx]
                    tiles[idx] = None
                    jj = jp - g * jg
                    ps2ap = bass.AP(
                        tensor=pt2.tensor,
                        offset=pt2.offset,
                        ap=[pt2.ap[0], [PSB, 2], [1, D_PAIR]],
                    )
                    drain(
                        code,
                        ps2ap,
                        ob[:, jj * D_PAIR : (jj + 2) * D_PAIR],
                        [jj, jj + 1],
                        pend,
                        2,
                    )

                def emit_stt(j):
                    jj = j - g * jg
                    s = ((j % L) // 8) * 2 + (j % 8 - 6)
                    nc.vector.scalar_tensor_tensor(
                        ob[:, jj * D_PAIR : (jj + 1) * D_PAIR],
                        wsepB_t,
                        sep_t[:, (j % L) : (j % L) + 1],
                        coB2_t[:, s * D_PAIR : (s + 1) * D_PAIR],
                        ALU.mult,
                        ALU.add,
                    )

                nstt = 0
                for idx, (code, jp) in enumerate(plan):
                    b = 64 * ((jp % L) // 128)
                    jo = jp % 128
                    pt2 = psj2.tile([IH, 2 * PSB], dt.float32, tag="p2", name="p2")
                    tiles[idx] = pt2
                    for t in range(2):
                        nc.tensor.matmul(
                            pt2[:, t * PSB : t * PSB + D_PAIR],
                            st2_t[b : b + 2, (jo + t) * IH : (jo + t + 1) * IH],
                            cw_t[b : b + 2, (jo + t) * D_PAIR : (jo + t + 1) * D_PAIR],
                            start=True,
                            stop=True,
                        )
                    if "nodrain" in vset:
                        continue
                    # one stt per pair slot keeps DVE fed with always-ready work
                    if nstt < len(stts) and idx % 3 == 0:
                        emit_stt(stts[nstt])
                        nstt += 1
                    if idx >= skew:
                        emit_drain(idx - skew)
                    if sum(len(v) for v in pend.values()) >= 4:
                        flush(pend, ob)
                    if nstt < len(stts) and idx % 3 == 2:
                        emit_stt(stts[nstt])
                        nstt += 1
                if "nodrain" in vset:
                    continue
                for idx in range(len(plan) - skew, len(plan)):
                    emit_drain(idx)
                while nstt < len(stts):
                    emit_stt(stts[nstt])
                    nstt += 1
                flush(pend, ob)
                if "nodma" not in vset:
                    eng = nc.scalar if (g % 2 == 1 and "altq" in vset) else nc.sync
                    eng.dma_start(out_d[:, g * jg : (g + 1) * jg, :], ob)
            return nc

        nb_g = jg // db
        nb = L // db
        seq = _quad_seq(pat, nb)

        def flush(pend, ob):
            """Coalesce pending cp-adds over contiguous j runs (width<=4)."""
            for eng_name, lst in pend.items():
                eng = nc.vector if eng_name == "dve" else nc.gpsimd
                lst.sort()
                run = []
                for jj in lst + [None]:
                    if run and (jj is None or jj != run[-1] + 1 or len(run) == 4):
                        w = len(run)
                        sl = ob[:, run[0] * D_PAIR : (run[0] + w) * D_PAIR]
                        eng.tensor_tensor(
                            sl, sl, cp4_t[:, 0 : w * D_PAIR], ALU.add
                        )
                        run = []
                    if jj is not None:
                        run.append(jj)
                lst.clear()

        for g in range(ngroups * repeat):
            g = g % ngroups
            ob = obp.tile([IH, jg * D_PAIR], dt.bfloat16, tag="ob", name="ob")
            pend = {"dve": [], "gps": []}
            for qd in range(nb_g):
                j0 = g * jg + qd * db
                code = seq[(j0 % L) // db]
                pt = psj.tile([IH, db * PSB], dt.float32, tag="ps", name="ps")
                for t in range(db):
                    j = j0 + t
                    b, jo = 64 * (j // 128), j % 128
                    nc.tensor.matmul(
                        pt[:, t * PSB : t * PSB + D_PAIR],
                        st2_t[b : b + 2, jo * IH : (jo + 1) * IH],
                        cw_t[b : b + 2, jo * D_PAIR : (jo + 1) * D_PAIR],
                        start=True,
                        stop=True,
                    )
                if "nodrain" in vset:
                    continue
                jj = j0 - g * jg
                obb = ob[:, jj * D_PAIR : (jj + db) * D_PAIR]
                cpb = cp4_t[:, 0 : db * D_PAIR]
                psb = bass.AP(
                    tensor=pt.tensor,
                    offset=pt.offset,
                    ap=[pt.ap[0], [PSB, db], [1, D_PAIR]],
                )
                if code == "P":
                    nc.vector.tensor_tensor(obb, psb, cpb, ALU.add)
                elif code == "Q":
                    nc.scalar.activation(obb, psb, AF.Identity)
                    pend["dve"].extend(range(jj, jj + db))
                elif code == "G":
                    nc.scalar.activation(obb, psb, AF.Identity)
                    pend["gps"].extend(range(jj, jj + db))
                elif code == "E":
                    nc.gpsimd.tensor_copy(obb, psb)
                    pend["dve"].extend(range(jj, jj + db))
                elif code == "F":
                    nc.gpsimd.tensor_copy(obb, psb)
                    pend["gps"].extend(range(jj, jj + db))
                else:  # D
                    nc.gpsimd.tensor_tensor(obb, psb, cpb, ALU.add)
                if sum(len(v) for v in pend.values()) >= 8:
                    flush(pend, ob)
            if "nodrain" not in vset:
                flush(pend, ob)
            if "nodma" not in vset and "nodrain" not in vset:
                eng = nc.sync if g % 2 == 0 else nc.scalar
                eng.dma_start(out_d[:, g * jg : (g + 1) * jg, :], ob)

    return nc


_NC_CACHE = []


def make_in_maps(seq, aa_idx, emb_table, W_proj, b_proj):
    seq = np.asarray(seq, dtype=np.int32)
    aa_idx = np.asarray(aa_idx, dtype=np.int32)
    emb_table = np.ascontiguousarray(np.asarray(emb_table, dtype=np.float32))
    W_proj = np.ascontiguousarray(np.asarray(W_proj, dtype=np.float32))
    b_proj = np.ascontiguousarray(np.asarray(b_proj, dtype=np.float32))
    in_maps = []
    for c in range(N_CORES):
        b, ih = c // 2, c % 2
        in_maps.append(
            {
                "seqb": np.ascontiguousarray(seq[b]),
                "seqi": np.ascontiguousarray(seq[b, ih * IH : (ih + 1) * IH]),
                "aab": np.ascontiguousarray(aa_idx[b]),
                "aai": np.ascontiguousarray(aa_idx[b, ih * IH : (ih + 1) * IH]),
                "emb": emb_table,
                "wp": W_proj,
                "bp": b_proj,
            }
        )
    return in_maps


def gather_out(results) -> np.ndarray:
    out = np.empty((B, L, L, D_PAIR), dtype=np.float32)
    for c in range(N_CORES):
        b, ih = c // 2, c % 2
        out[b, ih * IH : (ih + 1) * IH] = np.asarray(results[c]["out"]).astype(
            np.float32
        )
    return out


def kernel(seq, aa_idx, emb_table, W_proj, b_proj) -> np.ndarray:
    if not _NC_CACHE:
        nc = build()
        nc.finalize()
        _NC_CACHE.append(nc)
    nc = _NC_CACHE[0]
    in_maps = make_in_maps(seq, aa_idx, emb_table, W_proj, b_proj)
    res = run_bass_kernel_spmd(nc, in_maps, core_ids=list(range(N_CORES)))
    return gather_out(res.results)


# revision 24
# speedup vs baseline: 1.3450x; 1.0958x over previous
"""PairEmbedding Bass kernel for 8 TRN2 NeuronCores.

out[b,i,j,:] = Co[b,j,:] + Cp[b,i,:] + sep(b,i,j) * w_sep
  Co[j] = se_j @ W1 + [0 | pe_j]
  Cp[i] = se_i @ W2 + b_proj + [pe_i | 0]
  sep(i,j) = ln(|aa_i - aa_j| + 1)
where se = emb_table[seq], pe = pos_table[aa_idx], W1 = W_proj[0:144],
W2 = W_proj[144:288], w_sep = W_proj[288].

Sharding: core c -> batch b = c//2, row block i in [128*(c%2), 128*(c%2)+128),
all 256 j. Per-core output (128, 256, 288) stored bf16, host upcasts to f32
(bf16 rounding ~0.4% << the 2e-2 gate).

j-loop design (v1, cost-model-driven):
  Per j ONE K=2 matmul into PSUM: psum[i,d] = 1*co[j,d] + sep[i,j]*w[d]
  (stationary = [ones_row; sepT_row_j], moving = [co_row_j; w]).
  Drains are QUAD-batched (4 j's per instruction, strided read across 4
  half-banks of one PSUM tile) to amortize PSUM access latency:
    P: DVE   ob4 = ps4 + cp4                       (f32 psum read)
    Q: ACT   ob4 = Iden(ps4); DVE ob4 += cp4       (bf16 2x mode add)
    G: ACT   ob4 = Iden(ps4); GPS ob4 += cp4
    D: GPS   ob4 = ps4 + cp4                       (gpsimd psum read)
  DMA: groups of JG=32 j's (2.36 MB) alternating sync/scalar HWDGE queues.
Matmul operands indexed along the free dim from base partitions {0, 64}
(HW requires base partition in {0,32,64}); K=2 rows sit on partitions
{b, b+1}.
"""

import math
from contextlib import ExitStack

import numpy as np

from concourse import bacc, bass, mybir, tile
from concourse.bass_utils import run_bass_kernel_spmd

dt = mybir.dt
AF = mybir.ActivationFunctionType
ALU = mybir.AluOpType

B = 4
L = 256
D_PAIR = 288
D_HALF = 144
MAX_LEN = 260
VOCAB = 21
IH = 128          # i rows per core
JG = 32           # j's per output DMA group
N_CORES = 8
PSB = 512         # f32 slots per PSUM bank


def _pos_enc_table() -> np.ndarray:
    idx = np.arange(0, D_HALF, 2, dtype=np.float32)
    t = (np.float32(math.log(10000.0)) * idx) / np.float32(D_HALF)
    denom = np.exp(t, dtype=np.float32)
    pos = np.arange(MAX_LEN, dtype=np.float32)[:, None]
    pe = np.zeros((MAX_LEN, D_HALF), dtype=np.float32)
    pe[:, 0::2] = np.sin(pos / denom, dtype=np.float32)
    pe[:, 1::2] = np.cos(pos / denom, dtype=np.float32)
    return pe


def _bcast(ap_src, nparts: int):
    return bass.AP(
        tensor=ap_src.tensor, offset=ap_src.offset, ap=[[0, nparts], *ap_src.ap]
    )


def _quad_seq(pat: str, nquads: int):
    """'P:18,Q:30,G:6,D:10' (batch counts) -> interleaved length-nquads codes.
    'seq:PQDQ' -> the literal cycle repeated to length nquads."""
    if pat.startswith("seq:"):
        cyc = pat[4:]
        assert nquads % len(cyc) == 0, (cyc, nquads)
        return list(cyc) * (nquads // len(cyc))
    counts = {}
    for part in pat.split(","):
        k, v = part.split(":")
        counts[k.strip()] = int(v)
    assert sum(counts.values()) == nquads, (counts, nquads)
    seq = []
    used = {k: 0 for k in counts}
    for i in range(1, nquads + 1):
        k = max(counts, key=lambda c: counts[c] * i / nquads - used[c])
        seq.append(k)
        used[k] += 1
    return seq


def build(
    stage: str = "full",
    repeat: int = 1,
    variant: str = "",
    jg: int = 16,
    pat: str = "cell:QGQ,GQP,QGQ,GQP",
    obufs: int = 4,
    db: int = 4,
    psbufs: int = 0,
    skew: int = 5,
) -> bass.Bass:
    psbufs = psbufs or (4 if pat.startswith("cell:") else 8 // db)
    nc = bacc.Bacc("TRN2", target_bir_lowering=False)

    seqb_d = nc.dram_tensor("seqb", [L], dt.int32, kind="ExternalInput")
    seqi_d = nc.dram_tensor("seqi", [IH], dt.int32, kind="ExternalInput")
    aab_d = nc.dram_tensor("aab", [L], dt.int32, kind="ExternalInput")
    aai_d = nc.dram_tensor("aai", [IH], dt.int32, kind="ExternalInput")
    emb_d = nc.dram_tensor("emb", [VOCAB, D_HALF], dt.float32, kind="ExternalInput")
    wp_d = nc.dram_tensor("wp", [D_PAIR + 1, D_PAIR], dt.float32, kind="ExternalInput")
    bp_d = nc.dram_tensor("bp", [D_PAIR], dt.float32, kind="ExternalInput")
    out_d = nc.dram_tensor("out", [IH, L, D_PAIR], dt.bfloat16, kind="ExternalOutput")

    # pos-table gather sources, pre-arranged on host: chunk c of <=128 pos
    # rows on partitions, channel slice [0:144] (posL, pe_i) or [144:288]
    # (posR, pe_j), zero elsewhere.
    pos_np = _pos_enc_table()
    posL_np = np.zeros((128, 3 * D_PAIR), dtype=np.float32)
    posR_np = np.zeros((128, 3 * D_PAIR), dtype=np.float32)
    for c in range(3):
        rows = 128 if c < 2 else MAX_LEN - 256
        chunk = pos_np[c * 128 : c * 128 + rows, :]
        posL_np[0:rows, c * D_PAIR : c * D_PAIR + D_HALF] = chunk
        posR_np[0:rows, c * D_PAIR + D_HALF : (c + 1) * D_PAIR] = chunk
    posL_d = nc.inline_tensor(posL_np, "posL_c")
    posR_d = nc.inline_tensor(posR_np, "posR_c")
    iota_np = (
        np.arange(128, dtype=np.float32)[:, None]
        + 128.0 * np.arange(3, dtype=np.float32)[None, :]
    ).astype(np.float32)
    iota_d = nc.inline_tensor(iota_np, "iota")

    with tile.TileContext(nc) as tc, ExitStack() as ctx:
        persist = ctx.enter_context(tc.tile_pool(name="persist", bufs=1))

        # persistent tiles consumed by the j-loop.
        # st2: K=2 stationary rows. partition b   = ones (j block b),
        #                           partition b+1 = sepT rows: [jo*128+i] = sep[i, jo]
        st2_t = persist.tile([66, 128 * IH], dt.bfloat16, tag="st2")
        # cw:  K=2 moving rows.     partition b   = co rows: [jo*288+d] = co[jo, d]
        #                           partition b+1 = w_sep repeated per j
        cw_t = persist.tile([66, 128 * D_PAIR], dt.bfloat16, tag="cw")
        cp_t = persist.tile([IH, D_PAIR], dt.bfloat16, tag="cpt")
        cp4_t = persist.tile([IH, 4 * D_PAIR], dt.bfloat16, tag="cp4t")
        cell = pat.startswith("cell:")
        if cell:
            # coB2[i, s*288+d] = co[4s+3, d] + cp[i, d]: whole-column source
            # for PE-free stt j's (j % 4 == 3)
            coB2_t = persist.tile([IH, (L // 4) * D_PAIR], dt.bfloat16, tag="coB2")
            wsepB_t = persist.tile([IH, D_PAIR], dt.bfloat16, tag="wsepB")
            sep_t = persist.tile([IH, L], dt.float32, tag="sept")

        _qs = [nc.sync, nc.scalar]
        _qi = [0]

        def _q():
            _qi[0] ^= 1
            return _qs[_qi[0]]

        with ExitStack() as pre:
            scr = pre.enter_context(tc.tile_pool(name="scr", bufs=1))
            psc = pre.enter_context(tc.tile_pool(name="psc", bufs=1, space="PSUM"))

            # ---- input loads ----
            iota_t = scr.tile([128, 3], dt.float32, tag="iota")
            _q().dma_start(iota_t, iota_d[:, :])

            emb_t = scr.tile([VOCAB, D_HALF], dt.float32, tag="emb")
            _q().dma_start(emb_t, emb_d[:, :])

            w1a = scr.tile([128, D_PAIR], dt.float32, tag="w1a")
            _q().dma_start(w1a, wp_d[0:128, :])
            w1b = scr.tile([16, D_PAIR], dt.float32, tag="w1b")
            _q().dma_start(w1b, wp_d[128:144, :])
            w2a = scr.tile([128, D_PAIR], dt.float32, tag="w2a")
            _q().dma_start(w2a, wp_d[144:272, :])
            w2b = scr.tile([16, D_PAIR], dt.float32, tag="w2b")
            _q().dma_start(w2b, wp_d[272:288, :])

            wsep_f = scr.tile([1, D_PAIR], dt.float32, tag="wsepf")
            _q().dma_start(wsep_f, wp_d[288:289, :])

            bp_t = scr.tile([1, D_PAIR], dt.float32, tag="bp")
            _q().dma_start(bp_t, bp_d[:])

            aab_col = scr.tile([128, 2], dt.int32, tag="aabcol")
            _q().dma_start(
                aab_col,
                bass.AP(tensor=aab_d, offset=0, ap=[[1, 128], [128, 2]]),
            )
            aaB_i = scr.tile([128, L], dt.int32, tag="aaBi")
            _q().dma_start(aaB_i, _bcast(aab_d[:], 128))
            seqB_i = scr.tile([VOCAB, L], dt.int32, tag="seqBi")
            _q().dma_start(seqB_i, _bcast(seqb_d[:], VOCAB))
            seqI_i = scr.tile([VOCAB, IH], dt.int32, tag="seqIi")
            _q().dma_start(seqI_i, _bcast(seqi_d[:], VOCAB))
            aaIB_i = scr.tile([128, IH], dt.int32, tag="aaIBi")
            _q().dma_start(aaIB_i, _bcast(aai_d[:], 128))

            posL = scr.tile([128, 3 * D_PAIR], dt.float32, tag="posL")
            _q().dma_start(posL, posL_d[:, :])
            posR = scr.tile([128, 3 * D_PAIR], dt.float32, tag="posR")
            _q().dma_start(posR, posR_d[:, :])

            # ---- int -> f32 casts ----
            aab_colf = scr.tile([128, 2], dt.float32, tag="aabcolf")
            nc.vector.tensor_copy(aab_colf, aab_col)
            aaB_f = scr.tile([128, L], dt.float32, tag="aaBf")
            nc.vector.tensor_copy(aaB_f, aaB_i)
            seqB_f = scr.tile([VOCAB, L], dt.float32, tag="seqBf")
            nc.vector.tensor_copy(seqB_f, seqB_i)
            seqI_f = scr.tile([VOCAB, IH], dt.float32, tag="seqIf")
            nc.vector.tensor_copy(seqI_f, seqI_i)
            aaIB_f = scr.tile([128, IH], dt.float32, tag="aaIBf")
            nc.vector.tensor_copy(aaIB_f, aaIB_i)

            # ---- one-hots ----
            ohSeq = scr.tile([VOCAB, L], dt.float32, tag="ohSeq")
            nc.vector.tensor_scalar(
                ohSeq, seqB_f, iota_t[0:VOCAB, 0:1], None, ALU.is_equal
            )
            ohSeqI = scr.tile([VOCAB, IH], dt.float32, tag="ohSeqI")
            nc.vector.tensor_scalar(
                ohSeqI, seqI_f, iota_t[0:VOCAB, 0:1], None, ALU.is_equal
            )
            ohP = []
            ohPi = []
            for c in range(3):
                t = scr.tile([128, L], dt.float32, tag=f"ohP{c}", name=f"ohP{c}")
                nc.vector.tensor_scalar(t, aaB_f, iota_t[:, c : c + 1], None, ALU.is_equal)
                ohP.append(t)
                ti = scr.tile([128, IH], dt.float32, tag=f"ohPi{c}", name=f"ohPi{c}")
                nc.vector.tensor_scalar(
                    ti, aaIB_f, iota_t[:, c : c + 1], None, ALU.is_equal
                )
                ohPi.append(ti)

            # ---- seT = emb^T gathered by seq: (144, L) split 128+16 rows ----
            seT_a_ps = psc.tile([128, L], dt.float32, tag="seTaP")
            nc.tensor.matmul(seT_a_ps, emb_t[:, 0:128], ohSeq, start=True, stop=True)
            seT_b_ps = psc.tile([16, L], dt.float32, tag="seTbP")
            nc.tensor.matmul(
                seT_b_ps, emb_t[:, 128:D_HALF], ohSeq, start=True, stop=True
            )
            seT_a = scr.tile([128, L], dt.float32, tag="seTa")
            nc.vector.tensor_copy(seT_a, seT_a_ps)
            seT_b = scr.tile([16, L], dt.float32, tag="seTb")
            nc.vector.tensor_copy(seT_b, seT_b_ps)

            seTi_a_ps = psc.tile([128, IH], dt.float32, tag="seTiaP")
            nc.tensor.matmul(
                seTi_a_ps, emb_t[:, 0:128], ohSeqI, start=True, stop=True
            )
            seTi_b_ps = psc.tile([16, IH], dt.float32, tag="seTibP")
            nc.tensor.matmul(
                seTi_b_ps, emb_t[:, 128:D_HALF], ohSeqI, start=True, stop=True
            )
            seTi_a = scr.tile([128, IH], dt.float32, tag="seTia")
            nc.vector.tensor_copy(seTi_a, seTi_a_ps)
            seTi_b = scr.tile([16, IH], dt.float32, tag="seTib")
            nc.vector.tensor_copy(seTi_b, seTi_b_ps)

            # ---- Co halves -> bf16 -> flat rows at partitions 0 / 64 ----
            for h in range(2):
                co_ps = psc.tile(
                    [128, D_PAIR], dt.float32, tag=f"co{h}", name=f"co{h}"
                )
                sl = slice(h * 128, (h + 1) * 128)
                nc.tensor.matmul(co_ps, seT_a[:, sl], w1a, start=True, stop=False)
                nc.tensor.matmul(co_ps, seT_b[:, sl], w1b, start=False, stop=False)
                for c in range(3):
                    nc.tensor.matmul(
                        co_ps,
                        ohP[c][:, sl],
                        posR[:, c * D_PAIR : (c + 1) * D_PAIR],
                        start=False,
                        stop=(c == 2),
                    )
                co_bf = scr.tile(
                    [128, D_PAIR], dt.bfloat16, tag=f"cobf{h}", name=f"cobf{h}"
                )
                nc.vector.tensor_copy(co_bf, co_ps)
                _q().dma_start(cw_t[64 * h : 64 * h + 1, :], co_bf)

            # ---- Cp = se_i @ W2 + b_proj + [pe_i | 0], bf16 ----
            ones_f = scr.tile([1, IH], dt.float32, tag="onesf")
            nc.vector.memset(ones_f, 1.0)
            cp_ps = psc.tile([128, D_PAIR], dt.float32, tag="cpP")
            nc.tensor.matmul(cp_ps, seTi_a, w2a, start=True, stop=False)
            nc.tensor.matmul(cp_ps, seTi_b, w2b, start=False, stop=False)
            for c in range(3):
                nc.tensor.matmul(
                    cp_ps,
                    ohPi[c],
                    posL[:, c * D_PAIR : (c + 1) * D_PAIR],
                    start=False,
                    stop=False,
                )
            nc.tensor.matmul(cp_ps, ones_f, bp_t, start=False, stop=True)
            nc.vector.tensor_copy(cp_t, cp_ps)
            for r in range(4):
                nc.vector.tensor_copy(
                    cp4_t[:, r * D_PAIR : (r + 1) * D_PAIR], cp_t
                )

            # ---- sepT rows -> st2 partitions 1 / 65 ----
            for h in range(2):
                dist_t = scr.tile([128, IH], dt.float32, tag=f"dist{h}", name=f"dist{h}")
                nc.vector.tensor_scalar(
                    dist_t, aaIB_f, aab_colf[:, h : h + 1], None, ALU.subtract
                )
                abs_t = scr.tile([128, IH], dt.float32, tag=f"abs{h}", name=f"abs{h}")
                nc.scalar.activation(abs_t, dist_t, AF.Abs)
                sep_f = scr.tile([128, IH], dt.float32, tag=f"sepf{h}", name=f"sepf{h}")
                nc.scalar.activation(sep_f, abs_t, AF.Ln, bias=1.0)
                sep_bf = scr.tile(
                    [128, IH], dt.bfloat16, tag=f"sepbf{h}", name=f"sepbf{h}"
                )
                nc.vector.tensor_copy(sep_bf, sep_f)
                _q().dma_start(st2_t[64 * h + 1 : 64 * h + 2, :], sep_bf)

            # ---- ones rows -> st2 partitions 0 / 64 ----
            ones128 = scr.tile([128, IH], dt.bfloat16, tag="ones128")
            nc.vector.memset(ones128, 1.0)
            _q().dma_start(st2_t[0:1, :], ones128)
            _q().dma_start(st2_t[64:65, :], ones128)

            # ---- w_sep repeated rows -> cw partitions 1 / 65 ----
            wsep_bf = scr.tile([1, D_PAIR], dt.bfloat16, tag="wsepbf")
            nc.vector.tensor_copy(wsep_bf, wsep_f)
            wsrep_src = bass.AP(
                tensor=wsep_bf.tensor,
                offset=wsep_bf.offset,
                ap=[wsep_bf.ap[0], [0, 128], [1, D_PAIR]],
            )
            _q().dma_start(cw_t[1:2, :], wsrep_src)
            _q().dma_start(cw_t[65:66, :], wsrep_src)

            if cell:
                # ---- wsep broadcast rows + sep in i-partition layout ----
                wsepF_t = scr.tile([IH, D_PAIR], dt.float32, tag="wsepF")
                _q().dma_start(wsepF_t, _bcast(wp_d[288:289, :], 128))
                nc.vector.tensor_copy(wsepB_t, wsepF_t)
                aaCol_i = scr.tile([IH, 1], dt.int32, tag="aaColi")
                _q().dma_start(aaCol_i, aai_d[:])
                aaCol_f = scr.tile([IH, 1], dt.float32, tag="aaColf")
                nc.vector.tensor_copy(aaCol_f, aaCol_i)
                distI_t = scr.tile([IH, L], dt.float32, tag="distI")
                nc.vector.tensor_scalar(distI_t, aaB_f, aaCol_f, None, ALU.subtract)
                absI_t = scr.tile([IH, L], dt.float32, tag="absI")
                nc.scalar.activation(absI_t, distI_t, AF.Abs)
                nc.scalar.activation(sep_t, absI_t, AF.Ln, bias=1.0)

        if cell:
            # ---- coB2 = ones (x) co_row + cp, one slot per stt-j ----
            # stt j's are j % 8 in {6, 7}; slot s = (j//8)*2 + (j%8 - 6)
            with ExitStack() as pre2:
                ps2 = pre2.enter_context(
                    tc.tile_pool(name="ps2", bufs=4, space="PSUM")
                )
                for s in range(L // 4):
                    j = 8 * (s // 2) + 6 + (s % 2)
                    b, jo = 64 * (j // 128), j % 128
                    px = ps2.tile([IH, D_PAIR], dt.float32, tag="cob", name="cob")
                    nc.tensor.matmul(
                        px,
                        st2_t[b : b + 1, jo * IH : (jo + 1) * IH],
                        cw_t[b : b + 1, jo * D_PAIR : (jo + 1) * D_PAIR],
                        start=True,
                        stop=True,
                    )
                    nc.vector.tensor_tensor(
                        coB2_t[:, s * D_PAIR : (s + 1) * D_PAIR], px, cp_t, ALU.add
                    )

        if stage == "setup":
            dbg = ctx.enter_context(tc.tile_pool(name="dbg", bufs=1))
            dbf = dbg.tile([IH, D_PAIR], dt.bfloat16, tag="dbf")
            nc.vector.tensor_copy(dbf, cp_t)
            nc.sync.dma_start(out_d[:, 0:1, :], dbf)
            return nc

        # ---- j loop ----
        psj = ctx.enter_context(tc.tile_pool(name="psj", bufs=psbufs, space="PSUM"))
        obp = ctx.enter_context(tc.tile_pool(name="obp", bufs=obufs))
        ngroups = int(stage[5:]) if stage.startswith("jloop") else L // jg
        if variant == "dmaonly":
            obs = []
            for k in range(2):
                t = obp.tile([IH, jg * D_PAIR], dt.bfloat16, tag="ob", name="ob")
                nc.vector.memset(t, 0.5)
                obs.append(t)
            for g in range(ngroups * repeat):
                g = g % ngroups
                eng = nc.sync if g % 2 == 0 else nc.scalar
                eng.dma_start(out_d[:, g * jg : (g + 1) * jg, :], obs[g % 2])
            return nc
        vset = set(variant.split("+")) if variant else set()
        if cell:
            cyc = pat[5:].split(",")
            ncells_g = jg // 8
            psj2 = ctx.enter_context(tc.tile_pool(name="psj2", bufs=4, space="PSUM"))

            def drain(code, psap, obsl, jlist, pend, w):
                cpb = cp4_t[:, 0 : w * D_PAIR]
                if code == "P":
                    nc.vector.tensor_tensor(obsl, psap, cpb, ALU.add)
                elif code == "Q":
                    nc.scalar.activation(obsl, psap, AF.Identity)
                    pend["dve"].extend(jlist)
                elif code == "G":
                    nc.scalar.activation(obsl, psap, AF.Identity)
                    pend["gps"].extend(jlist)
                elif code == "E":
                    nc.gpsimd.tensor_copy(obsl, psap)
                    pend["dve"].extend(jlist)
                elif code == "F":
                    nc.gpsimd.tensor_copy(obsl, psap)
                    pend["gps"].extend(jlist)
                else:  # D
                    nc.gpsimd.tensor_tensor(obsl, psap, cpb, ALU.add)

            def flush(pend, ob):
                for eng_name, lst in pend.items():
                    eng = nc.vector if eng_name == "dve" else nc.gpsimd
                    lst.sort()
                    run = []
                    for jj in lst + [None]:
                        if run and (
                            jj is None or jj != run[-1] + 1 or len(run) == 4
                        ):
                            w = len(run)
                            sl = ob[:, run[0] * D_PAIR : (run[0] + w) * D_PAIR]
                            eng.tensor_tensor(
                                sl, sl, cp4_t[:, 0 : w * D_PAIR], ALU.add
                            )
                            run = []
                        if jj is not None:
                            run.append(jj)
                    lst.clear()

            # skew: pairs of drain lag behind matmul emission
            for g in range(ngroups * repeat):
                g = g % ngroups
                ob = obp.tile([IH, jg * D_PAIR], dt.bfloat16, tag="ob", name="ob")
                pend = {"dve": [], "gps": []}
                # flat pair list for the group: (code, j, tile placeholder)
                plan = []
                for cidx in range(ncells_g):
                    j0 = g * jg + cidx * 8
                    codes = cyc[((j0 % L) // 8) % len(cyc)]
                    for pidx in range(3):
                        plan.append((codes[pidx], j0 + 2 * pidx))
                stts = [g * jg + 8 * c + 6 + r for c in range(ncells_g) for r in range(2)]
                tiles = [None] * len(plan)

                def emit_drain(idx):
                    code, jp = plan[idx]
                    pt2 = tiles[idx]
                    tiles[idx] = None
                    jj = jp - g * jg
                    ps2ap = bass.AP(
                        tensor=pt2.tensor,
                        offset=pt2.offset,
                        ap=[pt2.ap[0], [PSB, 2], [1, D_PAIR]],
                    )
                    drain(
                        code,
                        ps2ap,
                        ob[:, jj * D_PAIR : (jj + 2) * D_PAIR],
                        [jj, jj + 1],
                        pend,
                        2,
                    )

                def emit_stt(j):
                    jj = j - g * jg
                    s = ((j % L) // 8) * 2 + (j % 8 - 6)
                    nc.vector.scalar_tensor_tensor(
                        ob[:, jj * D_PAIR : (jj + 1) * D_PAIR],
                        wsepB_t,
                        sep_t[:, (j % L) : (j % L) + 1],
                        coB2_t[:, s * D_PAIR : (s + 1) * D_PAIR],
                        ALU.mult,
                        ALU.add,
                    )

                nstt = 0
                for idx, (code, jp) in enumerate(plan):
                    b = 64 * ((jp % L) // 128)
                    jo = jp % 128
                    pt2 = psj2.tile([IH, 2 * PSB], dt.float32, tag="p2", name="p2")
                    tiles[idx] = pt2
                    for t in range(2):
                        nc.tensor.matmul(
                            pt2[:, t * PSB : t * PSB + D_PAIR],
                            st2_t[b : b + 2, (jo + t) * IH : (jo + t + 1) * IH],
                            cw_t[b : b + 2, (jo + t) * D_PAIR : (jo + t + 1) * D_PAIR],
                            start=True,
                            stop=True,
                        )
                    if "nodrain" in vset:
                        continue
                    # one stt per pair slot keeps DVE fed with always-ready work
                    if nstt < len(stts) and idx % 3 == 0 and "nostt" not in vset:
                        emit_stt(stts[nstt])
                        nstt += 1
                    if idx >= skew and "nodrain2" not in vset:
                        emit_drain(idx - skew)
                    if sum(len(v) for v in pend.values()) >= 4:
                        flush(pend, ob)
                    if nstt < len(stts) and idx % 3 == 2 and "nostt" not in vset:
                        emit_stt(stts[nstt])
                        nstt += 1
                if "nodrain" in vset:
                    continue
                if "nodrain2" not in vset:
                    for idx in range(len(plan) - skew, len(plan)):
                        emit_drain(idx)
                while nstt < len(stts) and "nostt" not in vset:
                    emit_stt(stts[nstt])
                    nstt += 1
                flush(pend, ob)
                if "nodma" not in vset:
                    eng = nc.scalar if (g % 2 == 1 and "altq" in vset) else nc.sync
                    eng.dma_start(out_d[:, g * jg : (g + 1) * jg, :], ob)
            return nc

        nb_g = jg // db
        nb = L // db
        seq = _quad_seq(pat, nb)

        def flush(pend, ob):
            """Coalesce pending cp-adds over contiguous j runs (width<=4)."""
            for eng_name, lst in pend.items():
                eng = nc.vector if eng_name == "dve" else nc.gpsimd
                lst.sort()
                run = []
                for jj in lst + [None]:
                    if run and (jj is None or jj != run[-1] + 1 or len(run) == 4):
                        w = len(run)
                        sl = ob[:, run[0] * D_PAIR : (run[0] + w) * D_PAIR]
                        eng.tensor_tensor(
                            sl, sl, cp4_t[:, 0 : w * D_PAIR], ALU.add
                        )
                        run = []
                    if jj is not None:
                        run.append(jj)
                lst.clear()

        for g in range(ngroups * repeat):
            g = g % ngroups
            ob = obp.tile([IH, jg * D_PAIR], dt.bfloat16, tag="ob", name="ob")
            pend = {"dve": [], "gps": []}
            for qd in range(nb_g):
                j0 = g * jg + qd * db
                code = seq[(j0 % L) // db]
                pt = psj.tile([IH, db * PSB], dt.float32, tag="ps", name="ps")
                for t in range(db):
                    j = j0 + t
                    b, jo = 64 * (j // 128), j % 128
                    nc.tensor.matmul(
                        pt[:, t * PSB : t * PSB + D_PAIR],
                        st2_t[b : b + 2, jo * IH : (jo + 1) * IH],
                        cw_t[b : b + 2, jo * D_PAIR : (jo + 1) * D_PAIR],
                        start=True,
                        stop=True,
                    )
                if "nodrain" in vset:
                    continue
                jj = j0 - g * jg
                obb = ob[:, jj * D_PAIR : (jj + db) * D_PAIR]
                cpb = cp4_t[:, 0 : db * D_PAIR]
                psb = bass.AP(
                    tensor=pt.tensor,
                    offset=pt.offset,
                    ap=[pt.ap[0], [PSB, db], [1, D_PAIR]],
                )
                if code == "P":
                    nc.vector.tensor_tensor(obb, psb, cpb, ALU.add)
                elif code == "Q":
                    nc.scalar.activation(obb, psb, AF.Identity)
                    pend["dve"].extend(range(jj, jj + db))
                elif code == "G":
                    nc.scalar.activation(obb, psb, AF.Identity)
                    pend["gps"].extend(range(jj, jj + db))
                elif code == "E":
                    nc.gpsimd.tensor_copy(obb, psb)
                    pend["dve"].extend(range(jj, jj + db))
                elif code == "F":
                    nc.gpsimd.tensor_copy(obb, psb)
                    pend["gps"].extend(range(jj, jj + db))
                else:  # D
                    nc.gpsimd.tensor_tensor(obb, psb, cpb, ALU.add)
                if sum(len(v) for v in pend.values()) >= 8:
                    flush(pend, ob)
            if "nodrain" not in vset:
                flush(pend, ob)
            if "nodma" not in vset and "nodrain" not in vset:
                eng = nc.sync if g % 2 == 0 else nc.scalar
                eng.dma_start(out_d[:, g * jg : (g + 1) * jg, :], ob)

    return nc


_NC_CACHE = []


def make_in_maps(seq, aa_idx, emb_table, W_proj, b_proj):
    seq = np.asarray(seq, dtype=np.int32)
    aa_idx = np.asarray(aa_idx, dtype=np.int32)
    emb_table = np.ascontiguousarray(np.asarray(emb_table, dtype=np.float32))
    W_proj = np.ascontiguousarray(np.asarray(W_proj, dtype=np.float32))
    b_proj = np.ascontiguousarray(np.asarray(b_proj, dtype=np.float32))
    in_maps = []
    for c in range(N_CORES):
        b, ih = c // 2, c % 2
        in_maps.append(
            {
                "seqb": np.ascontiguousarray(seq[b]),
                "seqi": np.ascontiguousarray(seq[b, ih * IH : (ih + 1) * IH]),
                "aab": np.ascontiguousarray(aa_idx[b]),
                "aai": np.ascontiguousarray(aa_idx[b, ih * IH : (ih + 1) * IH]),
                "emb": emb_table,
                "wp": W_proj,
                "bp": b_proj,
            }
        )
    return in_maps


def gather_out(results) -> np.ndarray:
    out = np.empty((B, L, L, D_PAIR), dtype=np.float32)
    for c in range(N_CORES):
        b, ih = c // 2, c % 2
        out[b, ih * IH : (ih + 1) * IH] = np.asarray(results[c]["out"]).astype(
            np.float32
        )
    return out


def kernel(seq, aa_idx, emb_table, W_proj, b_proj) -> np.ndarray:
    if not _NC_CACHE:
        nc = build()
        nc.finalize()
        _NC_CACHE.append(nc)
    nc = _NC_CACHE[0]
    in_maps = make_in_maps(seq, aa_idx, emb_table, W_proj, b_proj)
    res = run_bass_kernel_spmd(nc, in_maps, core_ids=list(range(N_CORES)))
    return gather_out(res.results)
